# revision 7
# baseline (speedup 1.0000x reference)
"""CenterLoss kernel v5 — baseline structure, tightened compute tail.

Differences from the 28.0us baseline:
  - subtracts write into one [128, 2048] diff tile so the Square+accum can
    run as 3 chunks (groups 0-3, 4-6, 7) instead of 8 ACT+ACCUM_READ pairs;
    the post-last-gather tail shrinks by ~0.7us.
  - x loaded via one linear-AP DMA per half (same as baseline).
  - output stays [128, 8] (4KB): empirically a smaller final DMA makes the
    completion-semaphore leg LONGER (1.5KB out measured 2.7-4.8us vs 4KB's
    1.7-2.4us — small transfers don't span all 16 DMA engines). Only
    columns 0..2 hold accumulator sums; the host ignores the rest.
"""

import numpy as np

from concourse import bacc, bass, mybir
import concourse.tile as tile
from concourse.bass_utils import run_bass_kernel_spmd

B = 8192
C = 10000
D = 256
N_CORES = 8
BL = B // N_CORES  # rows per core
P = 128            # SBUF partitions
G = BL // P        # row groups per core (8)

_CLIP_LO = 1e-12

_nc_cache = None


def _build():
    global _nc_cache
    if _nc_cache is not None:
        return _nc_cache

    # 64KB SWDGE scratch ring (default 16KB ~= 512 descriptor slots): the 8
    # gathers generate 1024 descriptors, so the default ring WRAPS mid-chain
    # and desc-gen stalls on slot reclaim when DMA consumption lags — seen as
    # +200-250ns on gathers 5-8 in slow runs. A ring that holds the whole
    # chain removes that stochastic stall mode.
    nc = bacc.Bacc(dynamic_dma_scratch_size=65536)
    x_l = nc.dram_tensor("x_local", [BL, D], mybir.dt.float32, kind="ExternalInput")
    lab_l = nc.dram_tensor("labels_local", [BL], mybir.dt.int32, kind="ExternalInput")
    cen = nc.dram_tensor("centers", [C, D], mybir.dt.float32, kind="ExternalInput")
    out = nc.dram_tensor("partials", [P, G], mybir.dt.float32, kind="ExternalOutput")

    with tile.TileContext(nc) as tc:
        with (
            tc.tile_pool(name="big", bufs=1) as big,
            tc.tile_pool(name="ctp", bufs=G) as ctp,
        ):
            lt = big.tile([P, G], mybir.dt.int32)
            xt = big.tile([P, G, D], mybir.dt.float32)
            df = big.tile([P, G * D], mybir.dt.float32)
            sq = big.tile([P, G * D], mybir.dt.float32)
            acc = big.tile([P, G], mybir.dt.float32)

            # labels first: the gather chain hangs off this DMA
            nc.sync.dma_start(out=lt[:], in_=lab_l[:].rearrange("(p g) -> p g", g=G))
            x_ap = x_l[:].rearrange("(p g) d -> p g d", g=G)
            nc.sync.dma_start(out=xt[:, 0:G // 2, :], in_=x_ap[:, 0:G // 2, :])
            nc.sync.dma_start(out=xt[:, G // 2:, :], in_=x_ap[:, G // 2:, :])

            for g in range(G):
                ct = ctp.tile([P, D], mybir.dt.float32, tag="ct")
                nc.gpsimd.indirect_dma_start(
                    out=ct[:],
                    out_offset=None,
                    in_=cen[:],
                    in_offset=bass.IndirectOffsetOnAxis(ap=lt[:, g:g + 1], axis=0),
                )
                nc.vector.tensor_sub(
                    out=df[:, g * D:(g + 1) * D], in0=xt[:, g, :], in1=ct[:]
                )
                if g == 3:
                    nc.scalar.activation(
                        out=sq[:, 0:4 * D],
                        in_=df[:, 0:4 * D],
                        func=mybir.ActivationFunctionType.Square,
                        accum_out=acc[:, 0:1],
                    )
                elif g == 6:
                    nc.scalar.activation(
                        out=sq[:, 4 * D:7 * D],
                        in_=df[:, 4 * D:7 * D],
                        func=mybir.ActivationFunctionType.Square,
                        accum_out=acc[:, 1:2],
                    )
            nc.scalar.activation(
                out=sq[:, 7 * D:8 * D],
                in_=df[:, 7 * D:8 * D],
                func=mybir.ActivationFunctionType.Square,
                accum_out=acc[:, 2:3],
            )
            nc.sync.dma_start(out=out[:], in_=acc[:])

    nc.finalize()
    _nc_cache = nc
    return nc


def _run(x, labels, centers, **spmd_kwargs):
    nc = _build()
    x = np.ascontiguousarray(np.asarray(x), dtype=np.float32)
    labels = np.ascontiguousarray(np.asarray(labels)).astype(np.int32)
    centers = np.ascontiguousarray(np.asarray(centers), dtype=np.float32)

    in_maps = []
    for c in range(N_CORES):
        sl = slice(c * BL, (c + 1) * BL)
        in_maps.append(
            {
                "x_local": x[sl],
                "labels_local": labels[sl],
                "centers": centers,
            }
        )
    res = run_bass_kernel_spmd(nc, in_maps, list(range(N_CORES)), **spmd_kwargs)
    partials = np.stack([r["partials"] for r in res.results])  # [8, P, G]
    # only accumulator columns 0..2 are meaningful; the rest is uninitialized
    loss = (
        partials[:, :, :3].astype(np.float64).sum() + B * (C - 1) * _CLIP_LO
    ) / B
    return np.asarray(loss, dtype=np.float32), res


def kernel(x, labels, centers):
    loss, _ = _run(x, labels, centers)
    return loss
